# revision 9
# baseline (speedup 1.0000x reference)
"""VQ codebook forward (nn_Codebook) on 8 Trainium2 NeuronCores.

Strategy (data-parallel over batch):
  - Shard x along batch: 1024 rows per core; replicate entries.
  - Host pre-transposes x and entries so the contraction dim (d) lands on
    SBUF partitions; host also precomputes -||e||^2 split into three bf16
    addends (b1+b2+b3 ~= -E2 to ~2^-25 rel) folded into the PE accumulation
    as a K=3 matmul against ones, so PSUM directly holds
        m[b, n] = 2*dot(x_b, e_n) - ||e_n||^2   (= -d2 + ||x_b||^2).
  - argmin_n d2 = argmax_n m: DVE `max` (top-8) + `max_index` per [128,1024]
    tile, both reading the PSUM accumulators directly.
  - Codebook lookup: SWDGE dma_gather of entry rows from HBM by the computed
    indices (round-tripped through a DRAM scratch to the wrapped-16 int16
    index layout the gather engine wants).
  - Losses: ||x-q||^2 = ||x||^2 - m*, so a single reduce of the per-row
    maxima (plus a host-side fp64 sum of x^2) yields both scalars.
Outputs are returned full-shape; x_q is the gathered entries (numerically
equal to x + stop_grad(q - x)).
"""

import numpy as np

BATCH, K, N, D = 8192, 8, 1024, 256
NCORES = 8
BSH = BATCH // NCORES          # batch rows per core
BT = BSH // 128                # 128-row tiles per core
P = 128
BETA = 0.25

_CACHE = {}


def _build_nc():
    import concourse.mybir as mybir
    from concourse import bacc
    from concourse.tile import TileContext

    f32 = mybir.dt.float32
    bf16 = mybir.dt.bfloat16
    u16 = mybir.dt.uint16
    i16 = mybir.dt.int16

    nc = bacc.Bacc("TRN2", target_bir_lowering=False, debug=False,
                   num_devices=NCORES)

    f16 = mybir.dt.float16
    xh = nc.dram_tensor("xh", [K, 2, P, BSH], f16, kind="ExternalInput")
    xl = nc.dram_tensor("xl", [K, 2, P, BSH], f16, kind="ExternalInput")
    eh = nc.dram_tensor("eh", [K, 2, P, N], f16, kind="ExternalInput")
    el = nc.dram_tensor("el", [K, 2, P, N], f16, kind="ExternalInput")
    ent = nc.dram_tensor("ent", [K, N, D], f32, kind="ExternalInput")
    b3 = nc.dram_tensor("b3", [K, 3, N], bf16, kind="ExternalInput")

    xq = nc.dram_tensor("xq", [K, P, BT, D], f32, kind="ExternalOutput")
    idxo = nc.dram_tensor("idxo", [K, P, BT], u16, kind="ExternalOutput")
    lsum = nc.dram_tensor("lsum", [P, 1], f32, kind="ExternalOutput")

    with TileContext(nc) as tc:
        with (
            tc.tile_pool(name="const", bufs=1) as cpool,
            tc.tile_pool(name="xp", bufs=4) as xpool,
            tc.tile_pool(name="qp", bufs=2) as qpool,
            tc.tile_pool(name="sp", bufs=2) as spool,
            tc.tile_pool(name="ps", bufs=4, space="PSUM") as ppool,
            tc.tile_pool(name="dr", bufs=2, space="DRAM") as dpool,
        ):
            # one-time loads
            eh_sb = cpool.tile([P, K, 2, N], f16)       # 4 MB
            el_sb = cpool.tile([P, K, 2, N], f16)       # 4 MB
            nc.sync.dma_start(eh_sb, eh.rearrange("k c d n -> d k c n"))
            nc.sync.dma_start(el_sb, el.rearrange("k c d n -> d k c n"))
            b3_sb = cpool.tile([3, K, N], bf16)
            nc.sync.dma_start(b3_sb, b3.rearrange("k t n -> t k n"))
            ones3 = cpool.tile([3, P], bf16)
            nc.vector.memset(ones3, 1.0)

            # persistent result accumulators
            mst = cpool.tile([P, K, BT, 8], f32)        # top-8 values per tile
            idx_all = cpool.tile([P, K, BT, 8], u16)    # top-8 indices per tile

            for k in range(K):
                for t in range(BT):
                    xh_sb = xpool.tile([P, 2, P], f16, tag="xh")
                    xl_sb = xpool.tile([P, 2, P], f16, tag="xl")
                    bsl = slice(t * P, (t + 1) * P)
                    nc.sync.dma_start(
                        xh_sb, xh[k, :, :, bsl].rearrange("c d b -> d c b"))
                    nc.sync.dma_start(
                        xl_sb, xl[k, :, :, bsl].rearrange("c d b -> d c b"))
                    ps = ppool.tile([P, N], mybir.dt.float32)
                    for h in range(2):
                        nh = slice(h * 512, (h + 1) * 512)
                        mms = [(lhs, rhs, c)
                               for lhs, rhs in ((xh_sb, eh_sb), (xh_sb, el_sb),
                                                (xl_sb, eh_sb))
                               for c in range(2)]
                        for i, (lhs, rhs, c) in enumerate(mms):
                            nc.tensor.matmul(ps[:, nh], lhsT=lhs[:, c],
                                             rhs=rhs[:, k, c, nh],
                                             start=(i == 0), stop=False)
                        nc.tensor.matmul(ps[:, nh], lhsT=ones3,
                                         rhs=b3_sb[:, k, nh],
                                         start=False, stop=True)
                    nc.vector.max(mst[:, k, t], ps)
                    nc.vector.max_index(idx_all[:, k, t], mst[:, k, t], ps)

                # indices of this k, in gather layout (int16, wrapped by 16)
                scr = dpool.tile([BSH], i16)
                with nc.allow_non_contiguous_dma(reason="tiny idx relayout"):
                    nc.sync.dma_start(
                        scr.rearrange("(t p) -> p t", p=P),
                        idx_all[:, k, :, 0].bitcast(i16),
                    )
                    gidx = spool.tile([P, BSH // 16], i16)
                    for r in range(8):
                        nc.sync.dma_start(
                            gidx[16 * r:16 * (r + 1)],
                            scr.rearrange("(s pp) -> pp s", pp=16),
                        )
                q_sb = qpool.tile([P, BT, D], f32)
                nc.gpsimd.dma_gather(
                    q_sb, ent[k], gidx,
                    num_idxs=BSH, num_idxs_reg=BSH, elem_size=D,
                )
                nc.sync.dma_start(xq[k], q_sb)
                nc.sync.dma_start(idxo[k], idx_all[:, k, :, 0])

            # loss partial: sum over (k, t) of per-row maxima
            lacc = cpool.tile([P, 1], f32)
            nc.vector.tensor_reduce(
                lacc, mst[:, :, :, 0:1], axis=mybir.AxisListType.XYZ,
                op=mybir.AluOpType.add,
            )
            nc.sync.dma_start(lsum[:], lacc)

    nc.compile()
    return nc


def _split16(a):
    h = a.astype(np.float16)
    l = (a.astype(np.float64) - h.astype(np.float64)).astype(np.float16)
    return h, l


def _prep_inputs(x, entries):
    """Host-side packing. Returns (shared_map, per_core maps, x2sum_f64)."""
    x = np.asarray(x, dtype=np.float32)
    entries = np.asarray(entries, dtype=np.float32)

    # eT [K, 2, 128, N]: contraction dim on partitions; fp16 hi/lo split
    eT = np.ascontiguousarray(
        entries.transpose(0, 2, 1).reshape(K, 2, P, N))
    eh, el = _split16(eT)

    # -||e||^2 in fp64, split into three bf16 addends
    import ml_dtypes
    e2 = (entries.astype(np.float64) ** 2).sum(axis=-1)      # [K, N]
    tgt = -e2
    b1 = tgt.astype(ml_dtypes.bfloat16)
    r1 = tgt - b1.astype(np.float64)
    b2 = r1.astype(ml_dtypes.bfloat16)
    r2 = r1 - b2.astype(np.float64)
    b3_ = r2.astype(ml_dtypes.bfloat16)
    b3 = np.ascontiguousarray(
        np.stack([b1, b2, b3_], axis=1))                     # [K, 3, N] bf16

    x2sum = float((x.astype(np.float64) ** 2).sum())

    x2 = 2.0 * x                                             # fold the "2" of 2*dot
    per_core = []
    for c in range(NCORES):
        xs = x2[c * BSH:(c + 1) * BSH]                       # [BSH, K, D]
        xT = np.ascontiguousarray(
            xs.transpose(1, 2, 0).reshape(K, 2, P, BSH))     # [K, 2, 128, BSH]
        xhc, xlc = _split16(xT)
        per_core.append({"xh": xhc, "xl": xlc})

    shared = {"eh": eh, "el": el, "ent": entries, "b3": b3}
    return shared, per_core, x2sum


def _run(nc, in_maps):
    from concourse.bass_utils import run_bass_kernel_spmd
    return run_bass_kernel_spmd(nc, in_maps, core_ids=list(range(NCORES)))


def kernel(x, entries):
    if "nc" not in _CACHE:
        _CACHE["nc"] = _build_nc()
    nc = _CACHE["nc"]

    shared, per_core, x2sum = _prep_inputs(x, entries)
    in_maps = [dict(shared, **per_core[c]) for c in range(NCORES)]
    res = _run(nc, in_maps)

    x_q = np.empty((BATCH, K, D), dtype=np.float32)
    idx = np.empty((BATCH, K), dtype=np.int32)
    msum = 0.0
    for c in range(NCORES):
        out = res.results[c]
        # xq [K, P, BT, D] -> [BSH(=t,p), K, D]
        x_q[c * BSH:(c + 1) * BSH] = (
            out["xq"].transpose(2, 1, 0, 3).reshape(BSH, K, D))
        idx[c * BSH:(c + 1) * BSH] = (
            out["idxo"].transpose(2, 1, 0).reshape(BSH, K).astype(np.int32))
        msum += float(out["lsum"].astype(np.float64).sum())

    mean = (x2sum - msum) / (BATCH * K)
    dict_loss = np.float32(mean)
    commit_loss = np.float32(0.25) * dict_loss

    return (x_q, idx, dict_loss, commit_loss)


# revision 12
# speedup vs baseline: 1.2065x; 1.2065x over previous
"""VQ codebook forward (nn_Codebook) on 8 Trainium2 NeuronCores.

Strategy (data-parallel over batch):
  - Shard x along batch: 1024 rows per core; replicate entries.
  - Host pre-transposes x and entries so the contraction dim (d) lands on
    SBUF partitions; host also precomputes -||e||^2 split into three bf16
    addends (b1+b2+b3 ~= -E2 to ~2^-25 rel) folded into the PE accumulation
    as a K=3 matmul against ones, so PSUM directly holds
        m[b, n] = 2*dot(x_b, e_n) - ||e_n||^2   (= -d2 + ||x_b||^2).
  - argmin_n d2 = argmax_n m: DVE `max` (top-8) + `max_index` per [128,1024]
    tile, both reading the PSUM accumulators directly.
  - Codebook lookup: SWDGE dma_gather of entry rows from HBM by the computed
    indices (round-tripped through a DRAM scratch to the wrapped-16 int16
    index layout the gather engine wants).
  - Losses: ||x-q||^2 = ||x||^2 - m*, so a single reduce of the per-row
    maxima (plus a host-side fp64 sum of x^2) yields both scalars.
Outputs are returned full-shape; x_q is the gathered entries (numerically
equal to x + stop_grad(q - x)).
"""

import numpy as np

BATCH, K, N, D = 8192, 8, 1024, 256
NCORES = 8
BSH = BATCH // NCORES          # batch rows per core
BT = BSH // 128                # 128-row tiles per core
P = 128
BETA = 0.25

_CACHE = {}


def _build_nc():
    import concourse.mybir as mybir
    from concourse import bacc
    from concourse.tile import TileContext

    f32 = mybir.dt.float32
    bf16 = mybir.dt.bfloat16
    u16 = mybir.dt.uint16
    i16 = mybir.dt.int16

    nc = bacc.Bacc("TRN2", target_bir_lowering=False, debug=False,
                   num_devices=NCORES)

    f16 = mybir.dt.float16
    xh = nc.dram_tensor("xh", [K, 2, P, BSH], f16, kind="ExternalInput")
    xl = nc.dram_tensor("xl", [K, 2, P, BSH], f16, kind="ExternalInput")
    eh = nc.dram_tensor("eh", [K, 2, P, N], f16, kind="ExternalInput")
    el = nc.dram_tensor("el", [K, 2, P, N], f16, kind="ExternalInput")
    ent = nc.dram_tensor("ent", [K, N, D], f32, kind="ExternalInput")
    b3 = nc.dram_tensor("b3", [K, 3, N], bf16, kind="ExternalInput")

    xq = nc.dram_tensor("xq", [K, P, BT, D], f32, kind="ExternalOutput")
    idxo = nc.dram_tensor("idxo", [K, P, BT], u16, kind="ExternalOutput")
    lsum = nc.dram_tensor("lsum", [P, 1], f32, kind="ExternalOutput")

    with TileContext(nc) as tc:
        with (
            tc.tile_pool(name="const", bufs=1) as cpool,
            tc.tile_pool(name="xp", bufs=4) as xpool,
            tc.tile_pool(name="qp", bufs=2) as qpool,
            tc.tile_pool(name="sp", bufs=2) as spool,
            tc.tile_pool(name="ps", bufs=4, space="PSUM") as ppool,
            tc.tile_pool(name="dr", bufs=2, space="DRAM") as dpool,
        ):
            # one-time loads; per-k pieces so k=0 compute starts early
            eh_sb = [cpool.tile([P, 2, N], f16, name=f"ehk{k}")
                     for k in range(K)]
            el_sb = [cpool.tile([P, 2, N], f16, name=f"elk{k}")
                     for k in range(K)]
            for k in range(K):
                nc.sync.dma_start(eh_sb[k], eh[k].rearrange("c d n -> d c n"))
                nc.sync.dma_start(el_sb[k], el[k].rearrange("c d n -> d c n"))
            b3_sb = cpool.tile([3, K, N], bf16)
            nc.sync.dma_start(b3_sb, b3.rearrange("k t n -> t k n"))
            ones3 = cpool.tile([3, P], bf16)
            nc.vector.memset(ones3, 1.0)

            # persistent result accumulators
            mst = cpool.tile([P, K, BT, 8], f32)        # top-8 values per tile
            idx_all = cpool.tile([P, K, BT, 8], u16)    # top-8 indices per tile

            for k in range(K):
                for t in range(BT):
                    xh_sb = xpool.tile([P, 2, P], f16, tag="xh")
                    xl_sb = xpool.tile([P, 2, P], f16, tag="xl")
                    bsl = slice(t * P, (t + 1) * P)
                    nc.sync.dma_start(
                        xh_sb, xh[k, :, :, bsl].rearrange("c d b -> d c b"))
                    nc.sync.dma_start(
                        xl_sb, xl[k, :, :, bsl].rearrange("c d b -> d c b"))
                    ps = ppool.tile([P, N], mybir.dt.float32)
                    for h in range(2):
                        nh = slice(h * 512, (h + 1) * 512)
                        mms = [(lhs, rhs, c)
                               for lhs, rhs in ((xh_sb, eh_sb[k]),
                                                (xh_sb, el_sb[k]),
                                                (xl_sb, eh_sb[k]))
                               for c in range(2)]
                        for i, (lhs, rhs, c) in enumerate(mms):
                            nc.tensor.matmul(ps[:, nh], lhsT=lhs[:, c],
                                             rhs=rhs[:, c, nh],
                                             start=(i == 0), stop=False)
                        nc.tensor.matmul(ps[:, nh], lhsT=ones3,
                                         rhs=b3_sb[:, k, nh],
                                         start=False, stop=True)
                    nc.vector.max(mst[:, k, t], ps)
                    nc.vector.max_index(idx_all[:, k, t], mst[:, k, t], ps)

                # indices of this k, in gather layout (int16, wrapped by 16)
                scr = dpool.tile([BSH], i16)
                with nc.allow_non_contiguous_dma(reason="tiny idx relayout"):
                    nc.sync.dma_start(
                        scr.rearrange("(t p) -> p t", p=P),
                        idx_all[:, k, :, 0].bitcast(i16),
                    )
                    gidx = spool.tile([P, BSH // 16], i16)
                    for r in range(8):
                        nc.sync.dma_start(
                            gidx[16 * r:16 * (r + 1)],
                            scr.rearrange("(s pp) -> pp s", pp=16),
                        )
                q_sb = qpool.tile([P, BT, D], f32)
                nc.gpsimd.dma_gather(
                    q_sb, ent[k], gidx,
                    num_idxs=BSH, num_idxs_reg=BSH, elem_size=D,
                )
                nc.scalar.dma_start(xq[k], q_sb)
                nc.sync.dma_start(idxo[k], idx_all[:, k, :, 0])

            # loss partial: sum over (k, t) of per-row maxima
            lacc = cpool.tile([P, 1], f32)
            nc.vector.tensor_reduce(
                lacc, mst[:, :, :, 0:1], axis=mybir.AxisListType.XYZ,
                op=mybir.AluOpType.add,
            )
            nc.sync.dma_start(lsum[:], lacc)

    nc.compile()
    return nc


def _split16(a):
    h = a.astype(np.float16)
    l = (a.astype(np.float64) - h.astype(np.float64)).astype(np.float16)
    return h, l


def _prep_inputs(x, entries):
    """Host-side packing. Returns (shared_map, per_core maps, x2sum_f64)."""
    x = np.asarray(x, dtype=np.float32)
    entries = np.asarray(entries, dtype=np.float32)

    # eT [K, 2, 128, N]: contraction dim on partitions; fp16 hi/lo split
    eT = np.ascontiguousarray(
        entries.transpose(0, 2, 1).reshape(K, 2, P, N))
    eh, el = _split16(eT)

    # -||e||^2 in fp64, split into three bf16 addends
    import ml_dtypes
    e2 = (entries.astype(np.float64) ** 2).sum(axis=-1)      # [K, N]
    tgt = -e2
    b1 = tgt.astype(ml_dtypes.bfloat16)
    r1 = tgt - b1.astype(np.float64)
    b2 = r1.astype(ml_dtypes.bfloat16)
    r2 = r1 - b2.astype(np.float64)
    b3_ = r2.astype(ml_dtypes.bfloat16)
    b3 = np.ascontiguousarray(
        np.stack([b1, b2, b3_], axis=1))                     # [K, 3, N] bf16

    x2sum = float((x.astype(np.float64) ** 2).sum())

    x2 = 2.0 * x                                             # fold the "2" of 2*dot
    per_core = []
    for c in range(NCORES):
        xs = x2[c * BSH:(c + 1) * BSH]                       # [BSH, K, D]
        xT = np.ascontiguousarray(
            xs.transpose(1, 2, 0).reshape(K, 2, P, BSH))     # [K, 2, 128, BSH]
        xhc, xlc = _split16(xT)
        per_core.append({"xh": xhc, "xl": xlc})

    shared = {"eh": eh, "el": el, "ent": entries, "b3": b3}
    return shared, per_core, x2sum


def _run(nc, in_maps):
    from concourse.bass_utils import run_bass_kernel_spmd
    return run_bass_kernel_spmd(nc, in_maps, core_ids=list(range(NCORES)))


def kernel(x, entries):
    if "nc" not in _CACHE:
        _CACHE["nc"] = _build_nc()
    nc = _CACHE["nc"]

    shared, per_core, x2sum = _prep_inputs(x, entries)
    in_maps = [dict(shared, **per_core[c]) for c in range(NCORES)]
    res = _run(nc, in_maps)

    x_q = np.empty((BATCH, K, D), dtype=np.float32)
    idx = np.empty((BATCH, K), dtype=np.int32)
    msum = 0.0
    for c in range(NCORES):
        out = res.results[c]
        # xq [K, P, BT, D] -> [BSH(=t,p), K, D]
        x_q[c * BSH:(c + 1) * BSH] = (
            out["xq"].transpose(2, 1, 0, 3).reshape(BSH, K, D))
        idx[c * BSH:(c + 1) * BSH] = (
            out["idxo"].transpose(2, 1, 0).reshape(BSH, K).astype(np.int32))
        msum += float(out["lsum"].astype(np.float64).sum())

    mean = (x2sum - msum) / (BATCH * K)
    dict_loss = np.float32(mean)
    commit_loss = np.float32(0.25) * dict_loss

    return (x_q, idx, dict_loss, commit_loss)
